# revision 87
# baseline (speedup 1.0000x reference)
"""Trainium2 Bass kernel for nn_Attention (B=2, T=2048, E=1024, H=16, D=64).

Sharding: 2 heads per core across 8 cores (tensor-parallel over heads).
Each core computes Q/K/V projections for its 2 heads, causal attention,
and a partial out-projection (its 128 feature columns of Wo); the host
sums the 8 partial outputs.

Key structure:
- S matmuls run with K=128 (full PE-array rate): q^T is stored as two
  zero-padded copies (one per head, the other head's 64 rows zeroed)
  so the full-partition k^T rhs needs no per-head slicing.
- Exact ONLINE softmax over [128, 1024] two-bank PSUM tiles: tile0 is
  exp'd immediately with its own exact row max (p <= 1, always safe;
  its PSUM frees at once); when tile1's max arrives, tile0's P is
  rescaled in SBUF by e^(m0-m) (4x-rate tensor_scalar) and the row
  sums combine. Wide single-instruction max/exp/accum halve the DVE /
  ACT fixed overheads; short cross-engine chains keep taus pipelined.
- PV: u-outer / head-inner; the two heads' matmuls target disjoint
  column-groups (tile_position) and execute concurrently (~2x).
- P^T transposes on the DMA XBAR into a j-major layout (each tau's
  transpose writes one contiguous region); PV reads a strided rhs.
- Normalization (1/rowsum) built once per 512-row block: one
  reciprocal + one PE transpose + four K=8 selector matmuls produce
  the [f, t] reciprocal-broadcast tile for all 4 row-tiles; the
  per-tau normalize multiply drains one slot before its
  out-projection matmuls so the PE weight load never waits on it.
- Deferred thunk queues (projection chunks, PV matmuls, normalize /
  out-projection) drain into the S phases at fine granularity; only
  projection chunk 0 of batch 0 runs before attention starts. Tiny
  zero-accumulate "keep warm" matmuls prevent the PE activity monitor
  from re-throttling the clock during softmax-paced stretches.
"""

import os
import sys

sys.path.insert(0, "/opt/trn_rl_repo")

import numpy as np
import concourse.bass as bass
import concourse.mybir as mybir
import concourse.tile as tile
from concourse import bacc
from concourse import bass_utils
from concourse.masks import make_identity

f32 = mybir.dt.float32
fp16 = mybir.dt.float16
bf16 = mybir.dt.bfloat16
AF = mybir.ActivationFunctionType
ALU = mybir.AluOpType
AX = mybir.AxisListType

B, T, E, H, D = 2, 2048, 1024, 16, 64
HL = 2              # heads per core
F = HL * D          # local feature cols (128)
NT = T // 128       # 16 t-tiles per batch
NE = E // 128       # 8 e-tiles
NTB = T // 512      # 4 t-blocks per batch
N_CORES = 8
INV_S = 1.0 / float(np.sqrt(T))


def build_nc():
    nc = bacc.Bacc("TRN2", target_bir_lowering=False, debug=False,
                   num_devices=N_CORES)
    xt_d = nc.dram_tensor("xt", [B, E, T], fp16, kind="ExternalInput").ap()
    wq_d = nc.dram_tensor("wq", [E, F], fp16, kind="ExternalInput").ap()
    wk_d = nc.dram_tensor("wk", [E, F], fp16, kind="ExternalInput").ap()
    wv_d = nc.dram_tensor("wv", [E, F], fp16, kind="ExternalInput").ap()
    wot_d = nc.dram_tensor("wot", [F, E], fp16, kind="ExternalInput").ap()
    sel_d = nc.dram_tensor("sel", [8, 512], fp16, kind="ExternalInput").ap()
    out_d = nc.dram_tensor("out", [B, T, E], bf16, kind="ExternalOutput").ap()

    with tile.TileContext(nc) as tc:
        with tc.tile_pool(name="const", bufs=1) as cpool, \
             tc.tile_pool(name="xtp", bufs=2) as xtp, \
             tc.tile_pool(name="qkv", bufs=2) as qkvp, \
             tc.tile_pool(name="pp", bufs=3) as ppool, \
             tc.tile_pool(name="ptb", bufs=2) as ptbp, \
             tc.tile_pool(name="smallp", bufs=8) as smallp, \
             tc.tile_pool(name="outp", bufs=6) as outp, \
             tc.tile_pool(name="ps_s", bufs=3, space="PSUM") as ps_s, \
             tc.tile_pool(name="ps_a", bufs=1, space="PSUM") as ps_a, \
             tc.tile_pool(name="ps_o", bufs=1, space="PSUM") as ps_o:

            # ---- constants ----
            ident_f = cpool.tile([128, 128], f32)
            make_identity(nc, ident_f[:])
            ident_h = cpool.tile([128, 128], fp16)
            nc.vector.tensor_copy(ident_h[:], ident_f[:])
            mask_f = cpool.tile([128, 128], f32)
            nc.gpsimd.memset(mask_f[:], 0.0)
            nc.gpsimd.affine_select(
                out=mask_f[:], in_=mask_f[:], compare_op=ALU.is_ge,
                fill=-30000.0, base=0, pattern=[[-1, 128]], channel_multiplier=1)
            mask_h = cpool.tile([128, 128], fp16)
            nc.vector.tensor_copy(mask_h[:], mask_f[:])
            # head-selector for the normalize broadcast:
            # sel2[c, j*128 + f] = 1 iff c == 2*j + f//64
            sel2 = cpool.tile([8, 512], fp16)

            # ---- weights (DMAs issued with the x loads below, in
            # first-use order so the first projection starts ASAP) ----
            wq_s = cpool.tile([128, NE, F], fp16)
            wk_s = cpool.tile([128, NE, F], fp16)
            wv_s = cpool.tile([128, NE, F], fp16)
            wot_s = cpool.tile([128, E], fp16)
            nc.sync.dma_start(wq_s[:], wq_d.rearrange("(n p) f -> p n f", p=128))
            nc.sync.dma_start(wk_s[:], wk_d.rearrange("(n p) f -> p n f", p=128))
            nc.sync.dma_start(wv_s[:], wv_d.rearrange("(n p) f -> p n f", p=128))

            # deferred work queues: PV matmuls and per-tau normalize/
            # out-projection thunks of the previous t-block, interleaved
            # into later emission points so no engine sits idle
            pv_queue = []
            norm_queue = []

            def emit_pv(k):
                for _ in range(min(k, len(pv_queue))):
                    pv_queue.pop(0)()

            def emit_norm(k):
                for _ in range(min(k, len(norm_queue))):
                    norm_queue.pop(0)()

            # ---- projections: b0 inline; b1's chunks become deferred
            # thunks drained into the first attention blocks' S phases ----
            # qz_b[b][h]: zero-padded q^T — rows of head h hold q, the
            # other head's 64 rows are zero. This lets the S matmuls run
            # with K=128 (full PE array rate): the kT rhs keeps both
            # heads' rows and the zero weights kill the cross terms.
            qz_b, kT_b, vT_b, vn_b, xt_b = {}, {}, {}, {}, {}
            for b in range(B):
                xt_b[b] = xt_s = xtp.tile([128, NE, T], fp16,
                                          name=f"xt_{b}", tag="xt")
                for e in range(NE):
                    nc.sync.dma_start(
                        xt_s[:, e, :], xt_d[b, e * 128:(e + 1) * 128])
                if b == 0:
                    # lower-priority loads go behind batch 0's x
                    nc.sync.dma_start(wot_s[:], wot_d)
                    nc.sync.dma_start(sel2[:], sel_d)
                qz_b[b] = [qkvp.tile([128, T], fp16,
                                     name=f"qz_{b}_{h}", tag=f"qz{h}")
                           for h in range(HL)]
                nc.vector.memset(qz_b[b][0][64:128, :], 0.0)
                nc.vector.memset(qz_b[b][1][0:64, :], 0.0)
                kT_b[b] = qkvp.tile([128, T], fp16, name=f"kT_{b}", tag="kT")
                vT_b[b] = qkvp.tile([128, T], bf16, name=f"vT_{b}", tag="vT")
                vn_b[b] = qkvp.tile([128, NT, F], bf16,
                                    name=f"vn_{b}", tag="vn")

            def mk_proj(b, n, w_s, dst):
                def f():
                    ps = ps_s.tile([128, 1024], f32,
                                   name=f"prj_{b}_{n}_{w_s.name}",
                                   tag="s")[:, 0:512]
                    for e in range(NE):
                        nc.tensor.matmul(
                            ps[:], w_s[:, e, :],
                            xt_b[b][:, e, n * 512:(n + 1) * 512],
                            start=(e == 0), stop=(e == NE - 1))
                    ns = slice(n * 512, (n + 1) * 512)
                    # scalar has more slack than vector in the S
                    # phases these drain into
                    eng_copy = nc.scalar.copy
                    if w_s is wq_s:
                        eng_copy(qz_b[b][0][0:64, ns], ps[0:64, :])
                        eng_copy(qz_b[b][1][64:128, ns], ps[64:128, :])
                    elif w_s is wv_s:
                        nc.vector.tensor_copy(dst[:, ns], ps[:])
                        # per-chunk V transpose so attention can start
                        # before the full projection finishes
                        nc.sync.dma_start_transpose(
                            vn_b[b][:, 4 * n:4 * n + 4, :],
                            vT_b[b][:, ns])
                    else:
                        eng_copy(dst[:, ns], ps[:])
                return f

            # only chunk 0 of batch 0 runs inline — it is all that
            # block (0, 0) needs; the rest drains into the S phases
            proj_queue = []
            for b in range(B):
                for n in range(T // 512):
                    for w_s, dst in ((wq_s, None), (wk_s, kT_b[b]),
                                     (wv_s, vT_b[b])):
                        th = mk_proj(b, n, w_s, dst)
                        if b == 0 and n == 0:
                            th()
                        else:
                            proj_queue.append(th)

            def emit_proj_q():
                k = 2 if len(proj_queue) > 8 else 1
                for _ in range(min(k, len(proj_queue))):
                    proj_queue.pop(0)()

            # ---- attention: alternate batches per block, largest block
            # first, so deferred PV/normalize work of each block drains
            # during the next block's softmax phase and the end-of-kernel
            # tail is the smallest block ----
            block_seq = [(0, 0), (0, 3), (1, 3), (0, 2),
                         (1, 2), (0, 1), (1, 1), (1, 0)]
            prev_a = [None]

            def keep_warm():
                # tiny zero-accumulate matmul (+0.0 into the previous
                # block's PV accumulator): keeps the PE activity
                # monitor from re-throttling the clock during
                # vector/scalar-paced softmax stretches
                if prev_a[0] is not None:
                    nc.tensor.matmul(
                        prev_a[0][0:64, 0:64], qz_b[0][1][0:64, 0:64],
                        kT_b[0][0:64, 0:64],
                        start=False, stop=False, skip_group_check=True)

            for b, tb in block_seq:
                qz, kT, vn = qz_b[b], kT_b[b], vn_b[b]
                # pt_blk[h]: P^T for this t-block, u-major:
                # [128 u-in-tile, u_tile, 512 t]
                # pt_blk[h]: P^T, j-major: [128 T-in-tile, j, u*128 + t]
                # — each tau's DMA transpose writes ONE contiguous
                # [128, L] region (fewer descriptors, full xbar BW);
                # the PV matmuls read a strided [128, 4, 128] rhs.
                pt_blk = [
                    ptbp.tile([128, 4, T], bf16,
                              name=f"ptb_{b}_{tb}_{h}", tag=f"ptb{h}")
                    for h in range(HL)]
                # row sums for the whole block: col index = 2*j + h
                lb_blk = smallp.tile([128, 8], f32,
                                     name=f"lb_{b}_{tb}", tag="lb", bufs=2)
                quota = (len(pv_queue) + 3) // 4 if pv_queue else 0
                unit = 0

                # zero the P^T regions above the causal diagonal so PV
                # can stream uniform 512-col blocks over all u-tiles
                for h in range(HL):
                    for jj in range(3):
                        lj = (tb * 4 + jj + 1) * 128
                        nc.gpsimd.memset(
                            pt_blk[h][:, jj, lj:(4 * tb + 4) * 128], 0.0)

                for j in range(4):
                    tau = tb * 4 + j
                    L = (tau + 1) * 128
                    nch = (L + 511) // 512
                    sml = {}
                    p_sb = {}
                    for h in range(HL):
                        sml[h] = smallp.tile(
                            [128, 12], f32,
                            name=f"sml_{b}_{tau}_{h}", tag="sml")
                        p_sb[h] = ppool.tile(
                            [128, T], bf16,
                            name=f"p_{b}_{tau}_{h}", tag="p")

                    # Online softmax over [128, 1024] two-bank PSUM
                    # tiles: tile0 is exp'd immediately with its own
                    # exact max (p <= 1, always safe, frees its PSUM
                    # right away); when tile1's max arrives, tile0's P
                    # is rescaled in SBUF by e^(m0 - m) — a cheap
                    # 4x-rate tensor_scalar — and the sums combine.
                    # Exact math, short cross-engine chains.
                    nt2 = (L + 1023) // 1024
                    # sml slots: 0=negm0, 1=negm1, 2=diff, 3=factor,
                    #            4=negm(global), 5=l0, 6=l1
                    s_tiles = {h: {} for h in range(HL)}

                    def emit_tile(h, c2):
                        t0 = c2 * 1024
                        w = min(1024, L - t0)
                        s_t = ps_s.tile(
                            [128, 1024], f32,
                            name=f"s_{b}_{tau}_{h}_{c2}", tag="s")
                        s_tiles[h][c2] = s_t
                        for cc in range((w + 511) // 512):
                            c0 = t0 + cc * 512
                            n = min(512, L - c0)
                            o0 = cc * 512
                            last = (c0 + n == L)
                            nc.tensor.matmul(
                                s_t[:, o0:o0 + n],
                                qz[h][:, tau * 128:(tau + 1) * 128],
                                kT[:, c0:c0 + n],
                                start=True, stop=not last)
                            if last:
                                # causal mask of the diag 128x128 block
                                nc.tensor.matmul(
                                    s_t[:, o0 + n - 128:o0 + n],
                                    ident_h[:], mask_h[:],
                                    start=False, stop=True)
                        nc.vector.reduce_max(
                            sml[h][:, c2:c2 + 1], s_t[:, :w], axis=AX.X,
                            negate=True)

                    for h in range(HL):
                        emit_tile(h, 0)
                    if pv_queue:
                        emit_pv(1)
                    else:
                        keep_warm()
                    for h in range(HL):
                        # exp tile0 with its own exact max; frees PSUM
                        nc.scalar.activation(
                            p_sb[h][:, 0:min(1024, L)],
                            s_tiles[h][0][:, :min(1024, L)],
                            AF.Exp, bias=sml[h][:, 0:1], scale=1.0,
                            accum_out=sml[h][:, 5:6])
                    if nt2 == 2:
                        for h in range(HL):
                            emit_tile(h, 1)
                        if pv_queue:
                            emit_pv(1)
                        else:
                            keep_warm()
                        for h in range(HL):
                            s_h = sml[h]
                            w = L - 1024
                            nc.vector.tensor_reduce(
                                s_h[:, 4:5], s_h[:, 0:2], axis=AX.X,
                                op=ALU.min)
                            nc.vector.tensor_tensor(
                                s_h[:, 2:3], s_h[:, 4:5], s_h[:, 0:1],
                                op=ALU.subtract)
                            nc.scalar.activation(
                                s_h[:, 3:4], s_h[:, 2:3], AF.Exp)
                            nc.scalar.activation(
                                p_sb[h][:, 1024:L],
                                s_tiles[h][1][:, :w],
                                AF.Exp, bias=s_h[:, 4:5], scale=1.0,
                                accum_out=s_h[:, 6:7])
                            # rescale tile0's P by e^(m0 - m) in SBUF
                            # (vector 4x mode: 16-bit, SBUF, single-src)
                            nc.vector.tensor_scalar_mul(
                                p_sb[h][:, 0:1024], p_sb[h][:, 0:1024],
                                s_h[:, 3:4])
                            # l = l0 * factor + l1
                            nc.vector.scalar_tensor_tensor(
                                lb_blk[:, 2 * j + h:2 * j + h + 1],
                                s_h[:, 5:6], s_h[:, 3:4], s_h[:, 6:7],
                                op0=ALU.mult, op1=ALU.add)
                            if h == 0 and pv_queue:
                                emit_pv(1)
                    else:
                        for h in range(HL):
                            nc.vector.tensor_copy(
                                lb_blk[:, 2 * j + h:2 * j + h + 1],
                                sml[h][:, 5:6])

                    for h in range(HL):
                        nc.sync.dma_start_transpose(
                            pt_blk[h][:, j, 0:L].rearrange(
                                "p (u q) -> p u q", q=128),
                            p_sb[h][:, 0:L])
                        emit_proj_q()
                        if unit < 3:
                            emit_pv(quota)
                        elif unit == 3:
                            emit_pv(len(pv_queue))
                        elif unit == 4:
                            emit_norm(2)
                        else:
                            emit_norm(1)
                        unit += 1

                # ---- queue PV for this block: A^T[f, 512 t] ----
                a_blk = ps_a.tile([128, 512], f32,
                                  name=f"a_{b}_{tb}", tag="a")
                u_last = 4 * tb + 3

                def mk_pv(u, tb=tb, a_blk=a_blk, pt_blk=pt_blk, vn=vn,
                          u_last=u_last):
                    def f():
                        for h in range(HL):
                            hr = slice(h * 64, (h + 1) * 64)
                            nc.tensor.matmul(
                                a_blk[hr, :], vn[:, u, hr],
                                pt_blk[h][:, 0:4,
                                          u * 128:(u + 1) * 128],
                                start=(u == 0), stop=(u == u_last),
                                tile_position=(0, h * 64),
                                skip_group_check=True)
                    return f

                for u in range(4 * (tb + 1)):
                    pv_queue.append(mk_pv(u))
                prev_a[0] = a_blk

                out_blk = [outp.tile([128, 2, E], bf16,
                                     name=f"ob_{b}_{tb}_{half}",
                                     tag="os", bufs=2)
                           for half in range(2)]
                rrep_sb = outp.tile([128, 512], fp16,
                                    name=f"rr_{b}_{tb}", tag="rr", bufs=2)

                def mk_rrep(tb=tb, b=b, lb_blk=lb_blk, rrep_sb=rrep_sb):
                    def f():
                        # rrep[f, j*128+t] = 1 / l[t, (j, h(f))] for the
                        # whole block, built with one transpose + one
                        # K=8 matmul
                        linv = smallp.tile([128, 8], f32,
                                           name=f"li_{b}_{tb}", tag="li")
                        nc.vector.reciprocal(linv[:], lb_blk[:])
                        t_ps = ps_o.tile([128, 512], f32,
                                         name=f"tp_{b}_{tb}", tag="o")
                        nc.tensor.transpose(
                            t_ps[0:8, 0:128], linv[:], ident_f[:])
                        lT = smallp.tile([8, 128], fp16,
                                         name=f"lt_{b}_{tb}", tag="lt")
                        nc.vector.tensor_copy(lT[:], t_ps[0:8, 0:128])
                        rr_ps = ps_o.tile([128, 512], f32,
                                          name=f"rp_{b}_{tb}", tag="o")
                        for j2 in range(4):
                            nc.tensor.matmul(
                                rr_ps[:, j2 * 128:(j2 + 1) * 128],
                                sel2[:, j2 * 128:(j2 + 1) * 128], lT[:],
                                start=True, stop=True)
                        nc.scalar.copy(rrep_sb[:], rr_ps[:])
                    return f

                at_box = {}

                def mk_at(j, tb=tb, a_blk=a_blk, b=b, rrep_sb=rrep_sb,
                          at_box=at_box):
                    def f():
                        tau = tb * 4 + j
                        js = slice(j * 128, (j + 1) * 128)
                        at_sb = smallp.tile(
                            [128, 128], fp16,
                            name=f"at_{b}_{tau}", tag="at", bufs=4)
                        at_box[j] = at_sb
                        nc.vector.tensor_tensor(
                            at_sb[:], a_blk[:, js],
                            rrep_sb[:, js], op=ALU.mult)
                    return f

                def mk_norm(j, tb=tb, b=b, out_blk=out_blk,
                            at_box=at_box):
                    def f():
                        tau = tb * 4 + j
                        at_sb = at_box[j]
                        ob = out_blk[j // 2]
                        for oc in range(2):
                            o_ps = ps_o.tile(
                                [128, 512], f32,
                                name=f"o_{b}_{tau}_{oc}", tag="o")
                            nc.tensor.matmul(
                                o_ps[:], at_sb[:],
                                wot_s[:, oc * 512:(oc + 1) * 512],
                                start=True, stop=True)
                            if oc == 0:
                                nc.vector.tensor_copy(
                                    ob[:, j % 2, 0:512], o_ps[:])
                            else:
                                nc.scalar.copy(
                                    ob[:, j % 2, 512:1024], o_ps[:])
                        if j % 2 == 1:
                            t0 = tb * 512 + (j // 2) * 256
                            nc.sync.dma_start(
                                out_d[b, t0:t0 + 256, :]
                                .rearrange("(jj p) e -> p jj e", p=128),
                                out_blk[j // 2][:, :, :])
                    return f

                # the at-mult of tau j drains one slot before its
                # out-projection matmuls, so the PE weight load for
                # the latter never waits on a just-finished vector op
                norm_queue.append(mk_rrep())
                norm_queue.append(mk_at(0))
                for j in range(4):
                    if j < 3:
                        def both(j=j, mn=mk_norm, ma=mk_at):
                            mn(j)()
                            ma(j + 1)()
                        norm_queue.append(both)
                    else:
                        norm_queue.append(mk_norm(j))

            # flush the final block's deferred work, interleaving the
            # tensor-bound PV pairs with the vector/scalar-bound
            # normalize chains so the tail overlaps across engines
            emit_norm(2)
            while pv_queue or norm_queue:
                emit_pv(2)
                emit_norm(1)

    nc.compile()
    return nc


_NC_CACHE = None


def _get_nc():
    global _NC_CACHE
    if _NC_CACHE is None:
        _NC_CACHE = build_nc()
    return _NC_CACHE


def make_in_maps(x, Wq, Wk, Wv, Wo):
    x = np.asarray(x, np.float32)
    Wq = np.asarray(Wq, np.float32)
    Wk = np.asarray(Wk, np.float32)
    Wv = np.asarray(Wv, np.float32)
    Wo = np.asarray(Wo, np.float32)
    xtr = np.ascontiguousarray(x.transpose(0, 2, 1))  # [B, E, T]
    xt = xtr.astype(np.float16)
    in_maps = []
    for c in range(N_CORES):
        h0 = c * HL
        wq = (np.concatenate([Wq[h0 + i] for i in range(HL)], axis=1)
              * np.float32(INV_S)).astype(np.float16)
        wk = np.concatenate([Wk[h0 + i] for i in range(HL)],
                            axis=1).astype(np.float16)
        wv = np.concatenate([Wv[h0 + i] for i in range(HL)],
                            axis=1).astype(np.float16)
        wot = np.ascontiguousarray(
            Wo[:, c * F:(c + 1) * F].T).astype(np.float16)
        # sel[c, j*128 + f] = 1 iff c == 2*j + f//64
        sel = np.zeros((8, 512), np.float16)
        for jj in range(4):
            for hh in range(2):
                sel[2 * jj + hh,
                    jj * 128 + hh * 64:jj * 128 + hh * 64 + 64] = 1.0
        in_maps.append({"xt": xt, "wq": wq, "wk": wk, "wv": wv,
                        "wot": wot, "sel": sel})
    return in_maps


def run_on_cores(in_maps, trace=False, **kw):
    nc = _get_nc()
    return bass_utils.run_bass_kernel_spmd(
        nc, in_maps, core_ids=list(range(N_CORES)), trace=trace, **kw)


def kernel(x, mask, Wq, Wk, Wv, Wo):
    # force the traceless PJRT path: the NTFF trace hook module is not
    # present in every environment, and grading only needs results
    os.environ["BASS_NEVER_TRACE"] = "1"
    in_maps = make_in_maps(x, Wq, Wk, Wv, Wo)
    res = run_on_cores(in_maps)
    acc = np.zeros((B, T, E), np.float32)
    for c in range(N_CORES):
        acc += np.asarray(res.results[c]["out"], dtype=np.float32)
    return acc
